# revision 1
# baseline (speedup 1.0000x reference)
"""Trainium2 Bass kernel: 3-layer GraphConv GNN + MLP heads, data-parallel over batch.

Contract: kernel(**inputs) takes the FULL unsharded numpy inputs (same keys as
setup_inputs()) and returns (pi, vf) full-shape numpy arrays.

Strategy (per the data-parallel sharding hint):
  - 8 NeuronCores, 128 batch elements each. Graph structure + weights replicated.
  - The fixed graph's gather/scatter is folded on host into a dense normalized
    adjacency A [256, 256]; aggregation becomes a dense matmul on TensorE.
  - Per-core pipeline alternates two SBUF layouts:
      P: [node (2x128 partitions), (batch, feat) free]
      Q: [(batch%4, feat) = 128 partitions, (quad, node) free]
    using A(H W) = (A H) W so each layer is:
      W-GEMM (Q->P, data-stationary, moving = blockdiag4(W))
      aggregation (P->Q, data-stationary, moving = A^T)
      bias+relu fused into the PSUM->SBUF copy (ScalarE activation / VectorE
      tensor_scalar), since Q puts features on partitions.
  - Layer-3's node-mean is fused into the ScalarE relu via accum_out.
  - Heads: emb GEMM + PE transposes to a [feat, batch] layout, then pi/vf are
    each one PSUM accumulation group of 3 matmuls (vec part, emb part, bias row).
  - Big matmuls use float32r with out free-dim >= 256 (1 cycle/row); tensors
    feeding fp32r matmuls are declared/produced as float32r end-to-end (walrus
    BIR verifier requirement). Tiny matmuls (transposes, bias rows, emb) stay
    plain fp32.
"""

import sys

import numpy as np

try:
    import concourse  # noqa: F401
except ImportError:  # pragma: no cover - fresh-dir fallback
    sys.path.insert(0, "/opt/trn_rl_repo")

import concourse.bacc as bacc
import concourse.bass as bass
import concourse.mybir as mybir
import concourse.tile as tile
from concourse.bass_utils import run_bass_kernel_spmd

F32 = mybir.dt.float32
F32R = mybir.dt.float32r
RELU = mybir.ActivationFunctionType.Relu
ADD = mybir.AluOpType.add
MAX = mybir.AluOpType.max
BF16 = mybir.dt.bfloat16

N_CORES = 8
B = 1024          # total batch
NB = B // N_CORES  # batches per core = 128
N = 256           # nodes
F8 = 8            # padded input feature dim (6 -> 8)
H = 32            # hidden feature dim
VEC = 26
DPI = 512
NQ = NB // 4      # quads per core = 32
NBLK = NB // 16   # 16-batch blocks per core = 8
Q_BUFS = 4
WP_BUFS = 4
STAGGER = True    # staggered For_i sem-reset: overlaps timing-loop iterations
HINTS = (mybir.EngineType.PE, mybir.EngineType.Activation, mybir.EngineType.DVE)
SPLIT_RELU = False


def build_nc(repeat: int = 1, use_for_i: bool = False) -> bacc.Bacc:
    """Build the per-core Bass program (SPMD: all cores run this)."""
    nc = bacc.Bacc("TRN2", target_bir_lowering=False, debug=False)

    # ---- DRAM I/O ----------------------------------------------------------
    x_d = nc.dram_tensor("x", [2, 128, NB * F8], F32R, kind="ExternalInput").ap()
    v_d = nc.dram_tensor("v", [NB, VEC], F32, kind="ExternalInput").ap()
    at_d = nc.dram_tensor("at", [2, 128, N], F32R, kind="ExternalInput").ap()
    w1sel_d = nc.dram_tensor("w1sel", [4, 128, 256], F32R, kind="ExternalInput").ap()
    wbd2_d = nc.dram_tensor("wbd2", [128, 256], F32R, kind="ExternalInput").ap()
    wbd3_d = nc.dram_tensor("wbd3", [128, 256], F32R, kind="ExternalInput").ap()
    wembbd_d = nc.dram_tensor("wembbd", [128, 256], F32, kind="ExternalInput").ap()
    bembp_d = nc.dram_tensor("bembp", [1, 256], F32, kind="ExternalInput").ap()
    biasr_d = nc.dram_tensor("biasr", [3, 128], F32, kind="ExternalInput").ap()
    ones_d = nc.dram_tensor("ones", [1, 128], F32, kind="ExternalInput").ap()
    ident_d = nc.dram_tensor("ident", [128, 128], F32, kind="ExternalInput").ap()
    wpiv_d = nc.dram_tensor("wpiv", [VEC, DPI], F32R, kind="ExternalInput").ap()
    wpie_d = nc.dram_tensor("wpie", [H, DPI], F32R, kind="ExternalInput").ap()
    bpi_d = nc.dram_tensor("bpi", [1, DPI], F32, kind="ExternalInput").ap()
    wvfv_d = nc.dram_tensor("wvfv", [VEC, DPI], F32R, kind="ExternalInput").ap()
    wvfe_d = nc.dram_tensor("wvfe", [H, DPI], F32R, kind="ExternalInput").ap()
    bvf_d = nc.dram_tensor("bvf", [1, DPI], F32, kind="ExternalInput").ap()
    pi_d = nc.dram_tensor("pi", [NB, DPI], F32, kind="ExternalOutput").ap()
    vf_d = nc.dram_tensor("vf", [NB, DPI], F32, kind="ExternalOutput").ap()

    with tile.TileContext(nc) as tc:
        with (
            tc.tile_pool(name="consts", bufs=1) as cp,
            tc.tile_pool(name="acts", bufs=1) as ap_,
            tc.tile_pool(name="xp", bufs=2) as xp_,
            tc.tile_pool(name="outs", bufs=2) as op_,
            tc.tile_pool(name="scratch", bufs=3) as sp_,
            tc.tile_pool(name="psum", bufs=1, space="PSUM") as pp_,
        ):
            # ---- constants (loaded once, outside the repeat loop) ----------
            at_sb = [cp.tile([128, N], F32R, tag=f"at{c}", name=f"at{c}") for c in range(2)]
            for c in range(2):
                nc.sync.dma_start(out=at_sb[c][:], in_=at_d[c])
            w1sel = [cp.tile([128, 256], F32R, tag=f"w1s{s}", name=f"w1s{s}") for s in range(4)]
            for s in range(4):
                nc.sync.dma_start(out=w1sel[s][:], in_=w1sel_d[s])
            wbd2 = cp.tile([128, 256], F32R, tag="wbd2")
            nc.sync.dma_start(out=wbd2[:], in_=wbd2_d[:])
            wbd3 = cp.tile([128, 256], F32R, tag="wbd3")
            nc.sync.dma_start(out=wbd3[:], in_=wbd3_d[:])
            wembbd = cp.tile([128, 256], F32, tag="wembbd")
            nc.sync.dma_start(out=wembbd[:], in_=wembbd_d[:])
            bembp = cp.tile([1, 256], F32, tag="bembp")
            nc.sync.dma_start(out=bembp[:], in_=bembp_d[:])
            biasr = [cp.tile([128, 1], F32, tag=f"b{l}r", name=f"b{l}r")
                     for l in range(3)]
            for l in range(3):
                nc.sync.dma_start(out=biasr[l][:], in_=biasr_d[l].unsqueeze(-1))
            ones1 = cp.tile([1, 128], F32, tag="ones1")
            nc.sync.dma_start(out=ones1[:], in_=ones_d[:])
            ident = cp.tile([128, 128], F32, tag="ident")
            nc.sync.dma_start(out=ident[:], in_=ident_d[:])
            wpiv = cp.tile([VEC, DPI], F32R, tag="wpiv")
            nc.sync.dma_start(out=wpiv[:], in_=wpiv_d[:])
            wpie = cp.tile([H, DPI], F32R, tag="wpie")
            nc.sync.dma_start(out=wpie[:], in_=wpie_d[:])
            bpi = cp.tile([1, DPI], F32, tag="bpi")
            nc.sync.dma_start(out=bpi[:], in_=bpi_d[:])
            wvfv = cp.tile([VEC, DPI], F32R, tag="wvfv")
            nc.sync.dma_start(out=wvfv[:], in_=wvfv_d[:])
            wvfe = cp.tile([H, DPI], F32R, tag="wvfe")
            nc.sync.dma_start(out=wvfe[:], in_=wvfe_d[:])
            bvf = cp.tile([1, DPI], F32, tag="bvf")
            nc.sync.dma_start(out=bvf[:], in_=bvf_d[:])

            def body():
                # ---- load inputs --------------------------------------
                xsb = [xp_.tile([128, NB * F8], F32R, tag=f"x{c}", name=f"x{c}") for c in range(2)]
                for c in range(2):
                    nc.sync.dma_start(out=xsb[c][:], in_=x_d[c])
                vsb = xp_.tile([NB, VEC], F32, tag="vsb")
                nc.sync.dma_start(out=vsb[:], in_=v_d[:])

                # Wait-slot discipline: the fp32/fp32r self-loading matmul has
                # ONE sync-wait slot (walrus S3_LW), enforced downstream by
                # Bacc.generate_event_semaphores -- but every split costs an
                # extra event-semaphore instruction, so the structure below
                # keeps matmul waits at <=1 by construction:
                #  - "gate" ldweights: absorb one producer wait on PE with no
                #    output (no PSUM WAW chains).
                #  - pair parity: all PSUM readers for psum-pair p run on
                #    engine p%2, and PSUM tags use even bufs so a recycled
                #    slot's previous reader is the same engine as the current
                #    pair's producer -> one combined wait.
                def gate(t):
                    nc.tensor.ldweights(t.bitcast(BF16))

                for c in range(2):
                    gate(xsb[c][:, 0:1])
                gate(vsb[:, 0:1])

                def relu_bias(par, dst, src, bias_ap):
                    if par % 2:
                        nc.scalar.activation(dst, src, RELU, bias=bias_ap)
                    else:
                        nc.vector.tensor_scalar(dst, src, bias_ap, 0.0, ADD, MAX)

                def plain_copy(par, dst, src):
                    if par % 2:
                        nc.scalar.copy(dst, src)
                    else:
                        nc.vector.tensor_copy(dst, src)

                # ---- L1 aggregation (P -> Q): Z1 = (A X)^T-ish ---------
                # z1 layout: [(b16, f8)=128, (blk, n')], blk-pairs share one
                # 512-col psum bank and one copy.
                z1 = ap_.tile([128, NBLK * N], F32R, tag="z1")
                for p in range(NBLK // 2):
                    q = pp_.tile([128, 2 * N], F32, tag="q", bufs=Q_BUFS, name="q1")
                    for half in range(2):
                        blk = 2 * p + half
                        for c in range(2):
                            nc.tensor.matmul(
                                q[:, half * N:(half + 1) * N],
                                xsb[c][:, blk * 128:(blk + 1) * 128],
                                at_sb[c][:], start=(c == 0), stop=(c == 1))
                    plain_copy(p, z1[:, p * 2 * N:(p + 1) * 2 * N], q[:])

                # ---- L1 W-GEMM (Q -> Q): h1 = relu(Z1 W1 + b1) ---------
                # h1 layout: [(b4, f)=128, (g, n')]; s-pairs share a bank.
                h1 = ap_.tile([128, NQ * N], F32R, tag="h1")
                for blk in range(NBLK):
                    gate(z1[:, blk * N:blk * N + 1])
                    for sp in range(2):
                        q = pp_.tile([128, 2 * N], F32, tag="wp", bufs=WP_BUFS, name="wq")
                        for half in range(2):
                            s = 2 * sp + half
                            nc.tensor.matmul(
                                q[:, half * N:(half + 1) * N], w1sel[s][:, :128],
                                z1[:, blk * N:(blk + 1) * N],
                                start=True, stop=True)
                        g0 = blk * 4 + 2 * sp
                        relu_bias(sp, h1[:, g0 * N:(g0 + 2) * N], q[:], biasr[0])

                # ---- L2 / L3 -------------------------------------------
                def layer(h_in, wbd, bias_ap, h_out):
                    # per quad: W-GEMM (Q->P, one 512-col bank, both n-chunks)
                    # then per quad-pair: aggregation (P->Q) into one bank.
                    y = sp_.tile([128, 2 * NQ * 128], F32R, tag="y", bufs=1)
                    y3 = y.rearrange("p (c q) -> p c q", c=2)
                    for q4 in range(4):
                        gate(h_in[:, q4 * N:q4 * N + 1])
                    for g in range(NQ):
                        w = pp_.tile([128, 512], F32, tag="wp", bufs=WP_BUFS, name="wp")
                        for c in range(2):
                            nc.tensor.matmul(
                                w[:, c * 256:(c + 1) * 256],
                                h_in[:, g * N + c * 128: g * N + (c + 1) * 128],
                                wbd[:], start=True, stop=True)
                        plain_copy(g // 2,
                                   y3[:, :, g * 128:(g + 1) * 128],
                                   w.rearrange("p (c j m) -> p c j m", c=2, m=128)[:, :, 0, :])
                    for p in range(NQ // 2):
                        if h_out is None:
                            gate(y[:, p * 256:p * 256 + 1])
                        q = pp_.tile([128, 2 * N], F32, tag="q", bufs=Q_BUFS, name="q2")
                        for half in range(2):
                            g = 2 * p + half
                            for c in range(2):
                                nc.tensor.matmul(
                                    q[:, half * N:(half + 1) * N],
                                    y3[:, c, g * 128:(g + 1) * 128],
                                    at_sb[c][:], start=(c == 0), stop=(c == 1))
                        dst = h_out if h_out is not None else h3
                        relu_bias(p, dst[:, p * 2 * N:(p + 1) * 2 * N], q[:], bias_ap)

                h2 = ap_.tile([128, NQ * N], F32R, tag="h2")
                layer(h1, wbd2, biasr[1], h2)
                h3 = ap_.tile([128, NQ * N], F32, tag="h3")
                layer(h2, wbd3, biasr[2], None)

                # ---- node-mean: hg[:, g] = sum_n h3[:, g, n] ------------
                # 4 big DVE reduces (axis X) over [128, 8, 256] views.
                hg = ap_.tile([128, NQ], F32, tag="hg")
                h3v = h3.rearrange("p (g n) -> p g n", n=N)
                for r in range(4):
                    nc.vector.tensor_reduce(
                        hg[:, r * 8:(r + 1) * 8].unsqueeze(-1),
                        h3v[:, r * 8:(r + 1) * 8, :],
                        mybir.AxisListType.X, ADD)

                # ---- emb = hg/256 @ W_emb + b_emb  (layout [g, (b4,e)]) -
                # plain fp32 matmuls (tiny); whole head stays on ACT.
                gate(hg[:, 0:1])
                ep = pp_.tile([32, 256], F32, tag="q", bufs=Q_BUFS, name="ep")
                nc.tensor.matmul(ep[:], hg[:], wembbd[:],
                                 start=True, stop=False)
                nc.tensor.matmul(ep[:], ones1[:, :NQ], bembp[:],
                                 start=False, stop=True)
                embg = sp_.tile([32, 128], F32, tag="embg")
                nc.scalar.copy(embg[:], ep[:, :128])

                # ---- transposes to feature-partition layouts ------------
                embf = sp_.tile([32, NB], F32R, tag="embf")
                embf_v = embf.rearrange("p (g c) -> p g c", c=4)
                for b4 in range(4):
                    tp = pp_.tile([32, 32], F32, tag="wp", bufs=WP_BUFS, name="tp")
                    nc.tensor.matmul(tp[:], embg[:, b4 * 32:(b4 + 1) * 32],
                                     ident[:32, :32], start=True, stop=True)
                    nc.scalar.copy(embf_v[:, :, b4], tp[:])
                vp = pp_.tile([VEC, NB], F32, tag="wp", bufs=WP_BUFS, name="vp")
                nc.tensor.matmul(vp[:], vsb[:], ident[:], start=True, stop=True)
                vf_t = sp_.tile([VEC, NB], F32R, tag="vft")
                nc.scalar.copy(vf_t[:], vp[:])

                # ---- heads ---------------------------------------------
                for wv, we, bb, out_d, tagn in (
                    (wpiv, wpie, bpi, pi_d, "pi"),
                    (wvfv, wvfe, bvf, vf_d, "vf"),
                ):
                    pp = pp_.tile([NB, DPI], F32, tag="wp", bufs=WP_BUFS, name=f"pp{tagn}")
                    nc.tensor.matmul(pp[:], vf_t[:], wv[:],
                                     start=True, stop=False)
                    nc.tensor.matmul(pp[:], embf[:], we[:],
                                     start=False, stop=False)
                    nc.tensor.matmul(pp[:], ones1[:], bb[:],
                                     start=False, stop=True)
                    osb = op_.tile([NB, DPI], F32, tag=f"o{tagn}", name=f"o{tagn}")
                    nc.scalar.activation(osb[:], pp[:], RELU)
                    nc.sync.dma_start(out=out_d[:], in_=osb[:])

            # one-time gates for every DMA-loaded matmul operand
            for t in (at_sb[0], at_sb[1], w1sel[0], w1sel[1], w1sel[2], w1sel[3],
                      wbd2, wbd3, wembbd, bembp, ones1, ident, wpiv, wpie, bpi,
                      wvfv, wvfe, bvf):
                nc.tensor.ldweights(t[0:1, 0:1].bitcast(BF16))

            if use_for_i and repeat > 1:
                with tc.For_i(0, repeat, 1, staggered_reset=STAGGER,
                              hint_engines=HINTS):
                    body()
            else:
                for _ in range(repeat):
                    body()

    nc.compile()
    return nc


# ---------------------------------------------------------------------------
# Host-side packing
# ---------------------------------------------------------------------------

def host_pack(inputs: dict) -> list[dict]:
    gf = np.ascontiguousarray(np.asarray(inputs["graph_feats"], dtype=np.float32))
    vec = np.ascontiguousarray(np.asarray(inputs["vector"], dtype=np.float32))
    src = np.asarray(inputs["src"]).astype(np.int64)
    dst = np.asarray(inputs["dst"]).astype(np.int64)
    W1 = np.asarray(inputs["W1"], dtype=np.float32)
    b1 = np.asarray(inputs["b1"], dtype=np.float32)
    W2 = np.asarray(inputs["W2"], dtype=np.float32)
    b2 = np.asarray(inputs["b2"], dtype=np.float32)
    W3 = np.asarray(inputs["W3"], dtype=np.float32)
    b3 = np.asarray(inputs["b3"], dtype=np.float32)
    W_emb = np.asarray(inputs["W_emb"], dtype=np.float32)
    b_emb = np.asarray(inputs["b_emb"], dtype=np.float32)
    W_pi = np.asarray(inputs["W_pi"], dtype=np.float32)
    b_pi = np.asarray(inputs["b_pi"], dtype=np.float32)
    W_vf = np.asarray(inputs["W_vf"], dtype=np.float32)
    b_vf = np.asarray(inputs["b_vf"], dtype=np.float32)

    # normalized dense adjacency (DGL GraphConv norm='both')
    deg_out = np.bincount(src, minlength=N).astype(np.float32)
    deg_in = np.bincount(dst, minlength=N).astype(np.float32)
    inv_o = np.where(deg_out > 0, deg_out ** -0.5, 0.0).astype(np.float32)
    inv_i = np.where(deg_in > 0, deg_in ** -0.5, 0.0).astype(np.float32)
    norm = inv_o[src] * inv_i[dst]
    A = np.zeros((N, N), dtype=np.float32)        # A[d, s]
    np.add.at(A, (dst, src), norm)
    AT = np.ascontiguousarray(A.T)                # AT[n, n'] = A[n', n]
    at_arr = AT.reshape(2, 128, N)

    # per-core X in [nchunk, n, b*8+f] layout
    gfp = np.zeros((B, N, F8), dtype=np.float32)
    gfp[:, :, :6] = gf

    # block-diag weight constants
    W1p = np.zeros((F8, H), dtype=np.float32)
    W1p[:6] = W1
    w1sel = np.zeros((4, 128, 256), dtype=np.float32)
    for s in range(4):
        for b4 in range(4):
            bb = s * 4 + b4
            w1sel[s, bb * F8:(bb + 1) * F8, b4 * H:(b4 + 1) * H] = W1p

    def blockdiag4(Wm):
        out = np.zeros((128, 256), dtype=np.float32)
        for b4 in range(4):
            out[b4 * H:(b4 + 1) * H, b4 * H:(b4 + 1) * H] = Wm
        return out

    wbd2 = blockdiag4(W2)
    wbd3 = blockdiag4(W3)
    wembbd = blockdiag4(W_emb / np.float32(N))
    bembp = np.zeros((1, 256), dtype=np.float32)
    bembp[0, :128] = np.tile(b_emb, 4)
    biasr = np.stack([np.tile(b, 4) for b in (b1, b2, b3)]).astype(np.float32)
    ones = np.ones((1, 128), dtype=np.float32)
    ident = np.eye(128, dtype=np.float32)
    wpiv = np.ascontiguousarray(W_pi[:VEC])
    wpie = np.ascontiguousarray(W_pi[VEC:])
    wvfv = np.ascontiguousarray(W_vf[:VEC])
    wvfe = np.ascontiguousarray(W_vf[VEC:])
    bpi = b_pi.reshape(1, DPI)
    bvf = b_vf.reshape(1, DPI)

    in_maps = []
    for c in range(N_CORES):
        gfc = gfp[c * NB:(c + 1) * NB]                      # [128, 256, 8]
        x = np.ascontiguousarray(gfc.transpose(1, 0, 2)).reshape(N, NB * F8)
        in_maps.append({
            "x": np.ascontiguousarray(x.reshape(2, 128, NB * F8)),
            "v": np.ascontiguousarray(vec[c * NB:(c + 1) * NB]),
            "at": at_arr, "w1sel": w1sel, "wbd2": wbd2, "wbd3": wbd3,
            "wembbd": wembbd, "bembp": bembp, "biasr": biasr, "ones": ones,
            "ident": ident, "wpiv": wpiv, "wpie": wpie, "bpi": bpi,
            "wvfv": wvfv, "wvfe": wvfe, "bvf": bvf,
        })
    return in_maps


_NC_CACHE: dict = {}


def kernel(**inputs):
    key = (1, False)
    if key not in _NC_CACHE:
        _NC_CACHE[key] = build_nc(*key)
    nc = _NC_CACHE[key]
    in_maps = host_pack(inputs)
    res = run_bass_kernel_spmd(nc, in_maps, list(range(N_CORES))).results
    pi = np.concatenate([res[c]["pi"] for c in range(N_CORES)], axis=0)
    vf = np.concatenate([res[c]["vf"] for c in range(N_CORES)], axis=0)
    return pi, vf

